# revision 19
# baseline (speedup 1.0000x reference)
"""Trainium2 Bass kernel for CrossEfficientAttention (B=8, C=256, H=W=64, 4 heads).

Sharding: data-parallel over batch B — one sample per NeuronCore, no collectives.

Per-core math (sample x_s, c_s of shape [C, N], N = H*W = 4096):
    Q  = wq @ x_s                      (+ bq, folded into the exp's ACT bias)
    KV = wkv @ c_s                     (bkv[:C] cancels exactly in softmax over N;
                                        bkv[C:] handled as a rank-1 update of W)
    k  = softmax_N(K); q = softmax_head(Q * C**-0.25)
    context = k @ V^T ; out = wo @ (context @ q) + bo

Restructured for the PE array (out = lhsT.T @ rhs, contraction over partitions):
  * KV^T computed directly in [N, C] layout by using c_s tiles as lhsT.
  * k-softmax normalizer: ones-columns appended to V^T give row sums of exp(K)
    in column 256 of the context PSUM accumulator; context rows are then scaled
    by the reciprocal column (per-partition tensor_scalar) — no transposes.
  * wo folded in early: W^T = matmul(lhsT=context, rhs=wo^T) directly in [d, o]
    layout. The per-chunk output is then just out2 = W^T.T @ q.
  * q-softmax denominators: block-indicator matmul sums exp(Q) per head into a
    [4, 512] PSUM tile; 1/D = exp(-ln D) on ScalarE (vector.reciprocal is
    8 cyc/elem, ACT Reciprocal is blocked); broadcast back to 128 partitions
    with a tiny selector matmul.

Matmuls run in float32r (single-pass PE, 4x faster than fp32 emulation).
Both loops are explicitly software-pipelined so the in-order PE queue never
waits on the ACT/DVE stages of the same iteration; weights ride in a single
packed DMA and output stores use the scalar-engine HWDGE queue to keep the
sync queue free for input streaming.
"""

import numpy as np

import concourse.bass as bass
import concourse.tile as tile
from concourse import bacc, mybir
from concourse.bass import ts
from concourse.bass_utils import run_bass_kernel_spmd

B, C, H, W = 8, 256, 64, 64
N = H * W
NHEADS = 4
DHEAD = C // NHEADS
NCORES = 8
NSUPER = N // 256          # 16 double-n-tile iterations for the KV phase
NCHUNKS = N // 512         # 8 column chunks for the Q/output phase
SCALE = float(1.0 / np.sqrt(np.sqrt(np.float32(C))))
VW = C + 2                 # V^T tile row width (256 data + 2 ones cols)
WP = 2 * C + C + C + NHEADS  # packed weight row width per c-half: wkvT|wqT|woT|ind

F32 = mybir.dt.float32
F32R = mybir.dt.float32r
AF = mybir.ActivationFunctionType

_CACHE = {}


def _single_act_table():
    """Scope-patch the activation-table list so the table-load pass resolves
    both Exp and Ln to natural_log_exp_and_others (set ids stay positional,
    so only the function lists may change, not the order)."""
    import contextlib

    import concourse.bacc as cbacc
    from concourse.hw_specs import get_activation_tables

    @contextlib.contextmanager
    def scope():
        orig = cbacc.get_activation_tables

        def patched(arch):
            tabs = get_activation_tables(arch)
            return {
                k: (v if k == "natural_log_exp_and_others" else set())
                for k, v in tabs.items()
            }

        cbacc.get_activation_tables = patched
        try:
            yield
        finally:
            cbacc.get_activation_tables = orig

    return scope()


def _build(use_bq, use_bo, use_bv, mm_dtype):
    nc = bacc.Bacc("TRN2", target_bir_lowering=False, debug=False)
    MDT = mm_dtype

    x = nc.dram_tensor("x", [C, N], MDT, kind="ExternalInput")
    cp = nc.dram_tensor("cp", [C, N], MDT, kind="ExternalInput")
    wpack = nc.dram_tensor("wpack", [128, 2 * WP], MDT, kind="ExternalInput")
    sel = nc.dram_tensor("sel", [NHEADS, C], MDT, kind="ExternalInput")
    if use_bq:
        bq_s = nc.dram_tensor("bq_s", [C, 1], F32, kind="ExternalInput")
    if use_bo:
        bo_c = nc.dram_tensor("bo_c", [C, 1], F32, kind="ExternalInput")
    if use_bv:
        bv_r = nc.dram_tensor("bv_r", [1, C], MDT, kind="ExternalInput")
        wosum = nc.dram_tensor("wosum", [1, C], MDT, kind="ExternalInput")
    y = nc.dram_tensor("y", [C, N], F32, kind="ExternalOutput")

    with tile.TileContext(nc) as tc:
        with (
            tc.tile_pool(name="const", bufs=1) as cst,
            tc.tile_pool(name="big", bufs=1) as big,
            tc.tile_pool(name="qsb", bufs=4) as qsb,
            tc.tile_pool(name="dsb", bufs=3) as dsb,
        ):
            # --- packed weights; the KV-phase slice (wkvT) rides first ---
            wpk = cst.tile([128, 2 * WP], MDT, name="wpk")
            wpk3 = wpk[:].rearrange("p (u w) -> p u w", u=2)
            wpack3 = wpack[:].rearrange("p (u w) -> p u w", u=2)
            wkvT_sb = [wpk[:, u * WP : u * WP + 2 * C] for u in range(2)]
            wqT_sb = [wpk[:, u * WP + 2 * C : u * WP + 3 * C] for u in range(2)]
            woT_sb = [wpk[:, u * WP + 3 * C : u * WP + 4 * C] for u in range(2)]
            ind_sb = [wpk[:, u * WP + 4 * C : u * WP + 4 * C + NHEADS] for u in range(2)]
            sel_sb = [cst.tile([NHEADS, 128], MDT, name=f"sel{u}") for u in range(2)]
            for u in range(2):
                nc.scalar.dma_start(out=sel_sb[u][:], in_=sel[:, ts(u, 128)])
            if use_bq:
                bq_sb = [cst.tile([128, 1], F32, name=f"bq{u}") for u in range(2)]
                for u in range(2):
                    nc.scalar.dma_start(out=bq_sb[u][:], in_=bq_s[ts(u, 128), :])
            if use_bo:
                bo_sb = [cst.tile([128, 1], F32, name=f"bo{u}") for u in range(2)]
                for u in range(2):
                    nc.scalar.dma_start(out=bo_sb[u][:], in_=bo_c[ts(u, 128), :])
            if use_bv:
                bv_sb = cst.tile([1, C], MDT, name="bv_sb")
                nc.scalar.dma_start(out=bv_sb[:], in_=bv_r[:])
                wosum_sb = cst.tile([1, C], MDT, name="wosum_sb")
                nc.scalar.dma_start(out=wosum_sb[:], in_=wosum[:])

            # --- sample loads: gate the first KV iterations on as little
            # data as possible, then stream the rest just ahead of use ---
            cf_sb = [big.tile([128, N], MDT, name=f"cf{u}") for u in range(2)]
            for u in range(2):
                nc.sync.dma_start(out=cf_sb[u][:, 0:512], in_=cp[ts(u, 128), 0:512])
            nc.sync.dma_start(out=wpk3[:, :, 0 : 2 * C], in_=wpack3[:, :, 0 : 2 * C])
            for c0, c1 in ((512, 1536), (1536, 2560), (2560, 3584), (3584, 4096)):
                for u in range(2):
                    nc.sync.dma_start(
                        out=cf_sb[u][:, c0:c1], in_=cp[ts(u, 128), c0:c1]
                    )
            nc.sync.dma_start(out=wpk3[:, :, 2 * C : WP], in_=wpack3[:, :, 2 * C : WP])
            xf_sb = [big.tile([128, N], MDT, name=f"xf{u}") for u in range(2)]
            for c0, c1 in ((0, 2048), (2048, 4096)):
                for u in range(2):
                    nc.sync.dma_start(
                        out=xf_sb[u][:, c0:c1], in_=x[ts(u, 128), c0:c1]
                    )

            # persistent W^T tiles (filled in the epilogue)
            WT_sb = [cst.tile([128, C], MDT, name=f"WT{u}") for u in range(2)]

            # HAM warmup: ~10 dependency-free matmuls on scratch data keep the
            # PE busy during the initial DMA wait so real matmuls start at
            # K=8/8 (2.4 GHz) instead of ramping from 1.2 GHz.
            scratch = cst.tile([128, 512], MDT, name="scratch")
            nc.vector.memset(scratch[:].bitcast(F32), 1.0)
            with tc.tile_pool(name="pswarm", bufs=1, space="PSUM") as pwm:
                pswarm = pwm.tile([128, 512], F32, name="pswarm")
                for _ in range(22):
                    nc.tensor.matmul(
                        pswarm[:], scratch[:, 0:128], scratch[:],
                        start=True, stop=True, skip_group_check=True,
                    )

            # manually-rotated V^T ring: ones columns pre-set once
            NVBUF = 4
            v2r = [cst.tile([128, 2 * VW], MDT, name=f"v2_{i}") for i in range(NVBUF)]
            for i in range(NVBUF):
                for h in range(2):
                    nc.vector.memset(
                        v2r[i][:, h * VW + C : h * VW + C + 2].bitcast(F32), 1.0
                    )

            eqs, psDs, rDs, psRbs, qts, psOs = {}, {}, {}, {}, {}, {}

            def q_mms_into(j, psQ):
                for t in range(2):
                    for u in range(2):
                        nc.tensor.matmul(
                            psQ[:, t * 512 : (t + 1) * 512],
                            wqT_sb[u][:, ts(t, 128)],
                            xf_sb[u][:, ts(j, 512)],
                            start=(u == 0),
                            stop=(u == 1),
                        )

            def eq_act(j, psQ):
                eq = qsb.tile([128, 1024], MDT, name="eq", tag="eq")
                if use_bq:
                    for t in range(2):
                        nc.scalar.activation(
                            out=eq[:, t * 512 : (t + 1) * 512],
                            in_=psQ[:, t * 512 : (t + 1) * 512],
                            func=AF.Exp,
                            scale=SCALE,
                            bias=bq_sb[t][:],
                        )
                else:
                    nc.scalar.activation(
                        out=eq[:], in_=psQ[:], func=AF.Exp, scale=SCALE
                    )
                eqs[j] = eq

            # ============ KV phase: context = exp(K) @ [V^T | 1] ============
            # Software-pipelined by one iteration: the PE runs iteration i's
            # KV matmuls and iteration i-1's context matmuls back to back.
            with tc.tile_pool(name="psum_ctx", bufs=1, space="PSUM") as pctx:
                psCtx = [
                    pctx.tile([128, C + 2], F32, name=f"psCtx{u}") for u in range(2)
                ]
                with (
                    tc.tile_pool(name="psum_kv", bufs=3, space="PSUM") as pkv,
                    tc.tile_pool(name="kvsb", bufs=3) as kvsb,
                ):
                    eks = {}

                    def kv_mms(i):
                        psKV = pkv.tile([128, 1024], F32, name="psKV")
                        for h in range(2):
                            nt = 2 * i + h
                            for u in range(2):
                                nc.tensor.matmul(
                                    psKV[:, h * 512 : (h + 1) * 512],
                                    cf_sb[u][:, ts(nt, 128)],
                                    wkvT_sb[u],
                                    start=(u == 0),
                                    stop=(u == 1),
                                )
                        return psKV

                    def ctx_mms(i):
                        ek = eks.pop(i)
                        v2 = v2r[i % NVBUF]
                        for h in range(2):
                            for u in range(2):
                                nc.tensor.matmul(
                                    psCtx[u][:],
                                    ek[:, h, ts(u, 128)],
                                    v2[:, h * VW : (h + 1) * VW],
                                    start=(i == 0 and h == 0),
                                    stop=(i == NSUPER - 1 and h == 1),
                                    skip_group_check=True,
                                )

                    def kv_post(i, psKV):
                        ek = kvsb.tile([128, 2, C], MDT, name="ek")
                        nc.scalar.activation(
                            out=ek[:],
                            in_=psKV[:].rearrange("p (h c) -> p h c", h=2)[:, :, 0:C],
                            func=AF.Exp,
                        )
                        eks[i] = ek
                        v2 = v2r[i % NVBUF]
                        nc.vector.tensor_copy(
                            v2[:].rearrange("p (h w) -> p h w", h=2)[:, :, 0:C],
                            psKV[:].rearrange("p (h c) -> p h c", h=2)[:, :, C : 2 * C],
                        )

                    for i in range(NSUPER):
                        psKV = kv_mms(i)
                        if i > 0:
                            ctx_mms(i - 1)
                        kv_post(i, psKV)
                    ctx_mms(NSUPER - 1)
                    # overlap the context epilogue with the first two Q chunks
                    # (their PSUM supertiles borrow the KV pool's slots)
                    for j in range(2):
                        psQ = pkv.tile([128, 1024], F32, name="psKV", tag="psKV")
                        q_mms_into(j, psQ)
                        eq_act(j, psQ)

                # ===== epilogue: normalize context, fold wo: W^T = ctx.T@woT =====
                rcol = [cst.tile([128, 1], F32, name=f"rcol{u}") for u in range(2)]
                ctx_sb = [cst.tile([128, C], MDT, name=f"ctx{u}") for u in range(2)]
                for u in range(2):
                    nc.vector.reciprocal(rcol[u][:], psCtx[u][:, C : C + 1])
                    nc.vector.tensor_scalar_mul(
                        out=ctx_sb[u][:], in0=psCtx[u][:, 0:C], scalar1=rcol[u][:]
                    )
                with tc.tile_pool(name="psum_w", bufs=1, space="PSUM") as pw:
                    psW = [pw.tile([128, C], F32, name=f"psW{v}") for v in range(2)]
                    for v in range(2):
                        for u in range(2):
                            nc.tensor.matmul(
                                psW[v][:],
                                ctx_sb[u][:, ts(v, 128)],
                                woT_sb[u],
                                start=(u == 0),
                                stop=(u == 1) and not use_bv,
                                skip_group_check=True,
                            )
                        if use_bv:
                            # context gains +bv[d'] per row (sum_n k = 1), so
                            # W^T += bv (X) rowsum(wo): a K=1 rank-1 matmul.
                            nc.tensor.matmul(
                                psW[v][:],
                                bv_sb[:, ts(v, 128)],
                                wosum_sb[:],
                                start=False,
                                stop=True,
                                skip_group_check=True,
                            )
                        nc.vector.tensor_copy(WT_sb[v][:], psW[v][:])

            # ============ Q phase: out = W^T.T @ softmax_head(exp(Q*s)) ============
            # Supertile layout [128, 1024]: channel-half t at cols 512t.
            # Pipelined depth 3: at iteration j the PE runs Q(j), D(j-1),
            # Rb(j-2), out(j-3) so every matmul's ACT/DVE inputs are a full
            # iteration old.
            with (
                tc.tile_pool(name="psq", bufs=1, space="PSUM") as pq,
                tc.tile_pool(name="psd", bufs=1, space="PSUM") as pd,
                tc.tile_pool(name="psrb", bufs=1, space="PSUM") as prb,
                tc.tile_pool(name="pso", bufs=1, space="PSUM") as po,
            ):
                def q_mms(j):
                    psQ = pq.tile([128, 1024], F32, name="psQ")
                    q_mms_into(j, psQ)
                    return psQ

                def d_mms(j):
                    psD = pd.tile([NHEADS, 512], F32, name="psD")
                    for t in range(2):
                        nc.tensor.matmul(
                            psD[:],
                            ind_sb[t],
                            eqs[j][:, t * 512 : (t + 1) * 512],
                            start=(t == 0),
                            stop=(t == 1),
                        )
                    psDs[j] = psD

                def r_acts(j):
                    lnD = dsb.tile([NHEADS, 512], F32, name="lnD")
                    nc.scalar.activation(out=lnD[:], in_=psDs.pop(j)[:], func=AF.Ln)
                    rD = dsb.tile([NHEADS, 512], MDT, name="rD")
                    nc.scalar.activation(out=rD[:], in_=lnD[:], func=AF.Exp, scale=-1.0)
                    rDs[j] = rD

                def rb_mms(j):
                    psRb = prb.tile([128, 1024], F32, name="psRb")
                    rD = rDs.pop(j)
                    for t in range(2):
                        nc.tensor.matmul(
                            psRb[:, t * 512 : (t + 1) * 512],
                            sel_sb[t][:],
                            rD[:],
                            start=True,
                            stop=True,
                        )
                    psRbs[j] = psRb

                def q_mul(j):
                    qt = qsb.tile([128, 1024], MDT, name="qt", tag="qt")
                    nc.vector.tensor_mul(qt[:], eqs.pop(j)[:], psRbs.pop(j)[:])
                    qts[j] = qt

                def out_mms(j):
                    psO = po.tile([128, 1024], F32, name="psO")
                    qt = qts.pop(j)
                    for t in range(2):
                        for u in range(2):
                            nc.tensor.matmul(
                                psO[:, t * 512 : (t + 1) * 512],
                                WT_sb[u][:, ts(t, 128)],
                                qt[:, u * 512 : (u + 1) * 512],
                                start=(u == 0),
                                stop=(u == 1),
                            )
                    psOs[j] = psO

                def store(j):
                    psO = psOs.pop(j)
                    o2 = qsb.tile([128, 1024], F32, name="o2", tag="o2")
                    if use_bo:
                        for t in range(2):
                            nc.vector.tensor_scalar_add(
                                out=o2[:, t * 512 : (t + 1) * 512],
                                in0=psO[:, t * 512 : (t + 1) * 512],
                                scalar1=bo_sb[t][:],
                            )
                    else:
                        nc.vector.tensor_copy(o2[:], psO[:])
                    for t in range(2):
                        nc.sync.dma_start(
                            out=y[ts(t, 128), ts(j, 512)],
                            in_=o2[:, t * 512 : (t + 1) * 512],
                        )

                for j in range(NCHUNKS + 3):
                    if 2 <= j < NCHUNKS:
                        psQ = q_mms(j)
                    if 1 <= j <= NCHUNKS:
                        d_mms(j - 1)
                    if 2 <= j <= NCHUNKS + 1:
                        rb_mms(j - 2)
                    if 3 <= j <= NCHUNKS + 2:
                        out_mms(j - 3)
                    if 2 <= j < NCHUNKS:
                        eq_act(j, psQ)
                    if 1 <= j <= NCHUNKS:
                        r_acts(j - 1)
                    if 2 <= j <= NCHUNKS + 1:
                        q_mul(j - 2)
                    if 3 <= j <= NCHUNKS + 2:
                        store(j - 3)

    nc.compile()
    return nc


def _get_nc(use_bq, use_bo, use_bv, mm_dtype):
    key = (use_bq, use_bo, use_bv, str(mm_dtype))
    if key not in _CACHE:
        with _single_act_table():
            _CACHE[key] = _build(use_bq, use_bo, use_bv, mm_dtype)
    return _CACHE[key]


def kernel(x, cproj, wq, bq, wkv, bkv, wo, bo, _mm_dtype=F32R, _results_hook=None):
    x = np.ascontiguousarray(np.asarray(x, dtype=np.float32).reshape(B, C, N))
    cf = np.ascontiguousarray(np.asarray(cproj, dtype=np.float32).reshape(B, C, N))
    wq = np.asarray(wq, dtype=np.float32)
    wkv = np.asarray(wkv, dtype=np.float32)
    wo = np.asarray(wo, dtype=np.float32)
    bq = np.asarray(bq, dtype=np.float32)
    bkv = np.asarray(bkv, dtype=np.float32)
    bo = np.asarray(bo, dtype=np.float32)

    use_bq = bool(np.any(bq != 0))
    use_bo = bool(np.any(bo != 0))
    bv = bkv[C:]
    use_bv = bool(np.any(bv != 0))

    wqT = np.ascontiguousarray(wq.T)
    wkvT = np.ascontiguousarray(wkv.T)
    woT = np.ascontiguousarray(wo.T)
    ind = np.zeros((C, NHEADS), np.float32)
    ind[np.arange(C), np.arange(C) // DHEAD] = 1.0
    sel = np.ascontiguousarray(ind.T)

    # packed weights: per c-half u: [wkvT | wqT | woT | ind] along the free dim
    wpack = np.zeros((128, 2 * WP), np.float32)
    for u in range(2):
        r = slice(u * 128, (u + 1) * 128)
        wpack[:, u * WP : u * WP + 2 * C] = wkvT[r]
        wpack[:, u * WP + 2 * C : u * WP + 3 * C] = wqT[r]
        wpack[:, u * WP + 3 * C : u * WP + 4 * C] = woT[r]
        wpack[:, u * WP + 4 * C : u * WP + 4 * C + NHEADS] = ind[r]

    nc = _get_nc(use_bq, use_bo, use_bv, _mm_dtype)

    base = {"wpack": wpack, "sel": sel}
    if use_bq:
        base["bq_s"] = (SCALE * bq).reshape(C, 1)
    if use_bo:
        base["bo_c"] = bo.reshape(C, 1)
    if use_bv:
        base["bv_r"] = bv.reshape(1, C)
        base["wosum"] = wo.sum(axis=1).reshape(1, C)

    in_maps = [dict(base, x=x[b], cp=cf[b]) for b in range(B)]
    res = run_bass_kernel_spmd(nc, in_maps, list(range(NCORES)))
    if _results_hook is not None:
        _results_hook(res)
    out = np.stack([res.results[b]["y"] for b in range(B)], axis=0)
    return out.reshape(B, C, H, W)


# revision 20
# speedup vs baseline: 2.6195x; 2.6195x over previous
"""Trainium2 Bass kernel for CrossEfficientAttention (B=8, C=256, H=W=64, 4 heads).

Sharding: data-parallel over batch B — one sample per NeuronCore, no collectives.

Per-core math (sample x_s, c_s of shape [C, N], N = H*W = 4096):
    Q  = wq @ x_s                      (+ bq, folded into the exp's ACT bias)
    KV = wkv @ c_s                     (bkv[:C] cancels exactly in softmax over N;
                                        bkv[C:] handled as a rank-1 update of W)
    k  = softmax_N(K); q = softmax_head(Q * C**-0.25)
    context = k @ V^T ; out = wo @ (context @ q) + bo

Restructured for the PE array (out = lhsT.T @ rhs, contraction over partitions):
  * KV^T computed directly in [N, C] layout by using c_s tiles as lhsT.
  * k-softmax normalizer: ones-columns appended to V^T give row sums of exp(K)
    in column 256 of the context PSUM accumulator; context rows are then scaled
    by the reciprocal column (per-partition tensor_scalar) — no transposes.
  * wo folded in early: W^T = matmul(lhsT=context, rhs=wo^T) directly in [d, o]
    layout. The per-chunk output is then just out2 = W^T.T @ q.
  * q-softmax denominators: block-indicator matmul sums exp(Q) per head into a
    [4, 512] PSUM tile; 1/D = exp(-ln D) on ScalarE (vector.reciprocal is
    8 cyc/elem, ACT Reciprocal is blocked); broadcast back to 128 partitions
    with a tiny selector matmul.

Matmuls run in float32r (single-pass PE, 4x faster than fp32 emulation).
Both loops are explicitly software-pipelined so the in-order PE queue never
waits on the ACT/DVE stages of the same iteration; weights ride in a single
packed DMA and output stores use the scalar-engine HWDGE queue to keep the
sync queue free for input streaming.
"""

import numpy as np

import concourse.bass as bass
import concourse.tile as tile
from concourse import bacc, mybir
from concourse.bass import ts
from concourse.bass_utils import run_bass_kernel_spmd

B, C, H, W = 8, 256, 64, 64
N = H * W
NHEADS = 4
DHEAD = C // NHEADS
NCORES = 8
NSUPER = N // 256          # 16 double-n-tile iterations for the KV phase
NCHUNKS = N // 512         # 8 column chunks for the Q/output phase
SCALE = float(1.0 / np.sqrt(np.sqrt(np.float32(C))))
VW = C + 2                 # V^T tile row width (256 data + 2 ones cols)
WP = 2 * C + C + C + NHEADS  # packed weight row width per c-half: wkvT|wqT|woT|ind

F32 = mybir.dt.float32
F32R = mybir.dt.float32r
AF = mybir.ActivationFunctionType

_CACHE = {}


def _single_act_table():
    """Scope-patch the activation-table list so the table-load pass resolves
    both Exp and Ln to natural_log_exp_and_others (set ids stay positional,
    so only the function lists may change, not the order)."""
    import contextlib

    import concourse.bacc as cbacc
    from concourse.hw_specs import get_activation_tables

    @contextlib.contextmanager
    def scope():
        orig = cbacc.get_activation_tables

        def patched(arch):
            tabs = get_activation_tables(arch)
            return {
                k: (v if k == "natural_log_exp_and_others" else set())
                for k, v in tabs.items()
            }

        cbacc.get_activation_tables = patched
        try:
            yield
        finally:
            cbacc.get_activation_tables = orig

    return scope()


def _build(use_bq, use_bo, use_bv, mm_dtype):
    nc = bacc.Bacc("TRN2", target_bir_lowering=False, debug=False)
    MDT = mm_dtype

    x = nc.dram_tensor("x", [C, N], MDT, kind="ExternalInput")
    cp = nc.dram_tensor("cp", [C, N], MDT, kind="ExternalInput")
    wpack = nc.dram_tensor("wpack", [128, 2 * WP], MDT, kind="ExternalInput")
    sel = nc.dram_tensor("sel", [NHEADS, C], MDT, kind="ExternalInput")
    if use_bq:
        bq_s = nc.dram_tensor("bq_s", [C, 1], F32, kind="ExternalInput")
    if use_bo:
        bo_c = nc.dram_tensor("bo_c", [C, 1], F32, kind="ExternalInput")
    if use_bv:
        bv_r = nc.dram_tensor("bv_r", [1, C], MDT, kind="ExternalInput")
        wosum = nc.dram_tensor("wosum", [1, C], MDT, kind="ExternalInput")
    y = nc.dram_tensor("y", [C, N], F32, kind="ExternalOutput")

    with tile.TileContext(nc) as tc:
        with (
            tc.tile_pool(name="const", bufs=1) as cst,
            tc.tile_pool(name="big", bufs=1) as big,
            tc.tile_pool(name="qsb", bufs=6) as qsb,
            tc.tile_pool(name="dsb", bufs=4) as dsb,
        ):
            # --- packed weights; the KV-phase slice (wkvT) rides first ---
            wpk = cst.tile([128, 2 * WP], MDT, name="wpk")
            wpk3 = wpk[:].rearrange("p (u w) -> p u w", u=2)
            wpack3 = wpack[:].rearrange("p (u w) -> p u w", u=2)
            wkvT_sb = [wpk[:, u * WP : u * WP + 2 * C] for u in range(2)]
            wqT_sb = [wpk[:, u * WP + 2 * C : u * WP + 3 * C] for u in range(2)]
            woT_sb = [wpk[:, u * WP + 3 * C : u * WP + 4 * C] for u in range(2)]
            ind_sb = [wpk[:, u * WP + 4 * C : u * WP + 4 * C + NHEADS] for u in range(2)]
            sel_sb = [cst.tile([NHEADS, 128], MDT, name=f"sel{u}") for u in range(2)]
            for u in range(2):
                nc.scalar.dma_start(out=sel_sb[u][:], in_=sel[:, ts(u, 128)])
            if use_bq:
                bq_sb = [cst.tile([128, 1], F32, name=f"bq{u}") for u in range(2)]
                for u in range(2):
                    nc.scalar.dma_start(out=bq_sb[u][:], in_=bq_s[ts(u, 128), :])
            if use_bo:
                bo_sb = [cst.tile([128, 1], F32, name=f"bo{u}") for u in range(2)]
                for u in range(2):
                    nc.scalar.dma_start(out=bo_sb[u][:], in_=bo_c[ts(u, 128), :])
            if use_bv:
                bv_sb = cst.tile([1, C], MDT, name="bv_sb")
                nc.scalar.dma_start(out=bv_sb[:], in_=bv_r[:])
                wosum_sb = cst.tile([1, C], MDT, name="wosum_sb")
                nc.scalar.dma_start(out=wosum_sb[:], in_=wosum[:])

            # --- sample loads: gate the first KV iterations on as little
            # data as possible, then stream the rest just ahead of use ---
            cf_sb = [big.tile([128, N], MDT, name=f"cf{u}") for u in range(2)]
            for u in range(2):
                nc.sync.dma_start(out=cf_sb[u][:, 0:512], in_=cp[ts(u, 128), 0:512])
            nc.sync.dma_start(out=wpk3[:, :, 0 : 2 * C], in_=wpack3[:, :, 0 : 2 * C])
            for c0, c1 in ((512, 1536), (1536, 2560), (2560, 3584), (3584, 4096)):
                for u in range(2):
                    nc.sync.dma_start(
                        out=cf_sb[u][:, c0:c1], in_=cp[ts(u, 128), c0:c1]
                    )
            nc.sync.dma_start(out=wpk3[:, :, 2 * C : WP], in_=wpack3[:, :, 2 * C : WP])
            xf_sb = [big.tile([128, N], MDT, name=f"xf{u}") for u in range(2)]
            for c0, c1 in ((0, 2048), (2048, 4096)):
                for u in range(2):
                    nc.sync.dma_start(
                        out=xf_sb[u][:, c0:c1], in_=x[ts(u, 128), c0:c1]
                    )

            # persistent W^T tiles (filled in the epilogue)
            WT_sb = [cst.tile([128, C], MDT, name=f"WT{u}") for u in range(2)]

            # HAM warmup: ~10 dependency-free matmuls on scratch data keep the
            # PE busy during the initial DMA wait so real matmuls start at
            # K=8/8 (2.4 GHz) instead of ramping from 1.2 GHz.
            scratch = cst.tile([128, 512], MDT, name="scratch")
            nc.vector.memset(scratch[:].bitcast(F32), 1.0)
            with tc.tile_pool(name="pswarm", bufs=1, space="PSUM") as pwm:
                pswarm = pwm.tile([128, 512], F32, name="pswarm")
                for _ in range(22):
                    nc.tensor.matmul(
                        pswarm[:], scratch[:, 0:128], scratch[:],
                        start=True, stop=True, skip_group_check=True,
                    )

            # manually-rotated V^T ring: ones columns pre-set once
            NVBUF = 4
            v2r = [cst.tile([128, 2 * VW], MDT, name=f"v2_{i}") for i in range(NVBUF)]
            for i in range(NVBUF):
                for h in range(2):
                    nc.vector.memset(
                        v2r[i][:, h * VW + C : h * VW + C + 2].bitcast(F32), 1.0
                    )

            eqs, psDs, rDs, psRbs, qts, psOs = {}, {}, {}, {}, {}, {}

            def q_mms_into(j, psQ):
                for t in range(2):
                    for u in range(2):
                        nc.tensor.matmul(
                            psQ[:, t * 512 : (t + 1) * 512],
                            wqT_sb[u][:, ts(t, 128)],
                            xf_sb[u][:, ts(j, 512)],
                            start=(u == 0),
                            stop=(u == 1),
                        )

            def eq_act(j, psQ):
                eq = qsb.tile([128, 1024], MDT, name="eq", tag="eq")
                if use_bq:
                    for t in range(2):
                        nc.scalar.activation(
                            out=eq[:, t * 512 : (t + 1) * 512],
                            in_=psQ[:, t * 512 : (t + 1) * 512],
                            func=AF.Exp,
                            scale=SCALE,
                            bias=bq_sb[t][:],
                        )
                else:
                    nc.scalar.activation(
                        out=eq[:], in_=psQ[:], func=AF.Exp, scale=SCALE
                    )
                eqs[j] = eq

            # ============ KV phase: context = exp(K) @ [V^T | 1] ============
            # Software-pipelined by one iteration: the PE runs iteration i's
            # KV matmuls and iteration i-1's context matmuls back to back.
            with tc.tile_pool(name="psum_ctx", bufs=1, space="PSUM") as pctx:
                psCtx = [
                    pctx.tile([128, C + 2], F32, name=f"psCtx{u}") for u in range(2)
                ]
                with (
                    tc.tile_pool(name="psum_kv", bufs=3, space="PSUM") as pkv,
                    tc.tile_pool(name="kvsb", bufs=3) as kvsb,
                ):
                    eks = {}

                    def kv_mms(i):
                        psKV = pkv.tile([128, 1024], F32, name="psKV")
                        for h in range(2):
                            nt = 2 * i + h
                            for u in range(2):
                                nc.tensor.matmul(
                                    psKV[:, h * 512 : (h + 1) * 512],
                                    cf_sb[u][:, ts(nt, 128)],
                                    wkvT_sb[u],
                                    start=(u == 0),
                                    stop=(u == 1),
                                )
                        return psKV

                    def ctx_mms(i):
                        ek = eks.pop(i)
                        v2 = v2r[i % NVBUF]
                        for h in range(2):
                            for u in range(2):
                                nc.tensor.matmul(
                                    psCtx[u][:],
                                    ek[:, h, ts(u, 128)],
                                    v2[:, h * VW : (h + 1) * VW],
                                    start=(i == 0 and h == 0),
                                    stop=(i == NSUPER - 1 and h == 1),
                                    skip_group_check=True,
                                )

                    def kv_post(i, psKV):
                        ek = kvsb.tile([128, 2, C], MDT, name="ek")
                        nc.scalar.activation(
                            out=ek[:],
                            in_=psKV[:].rearrange("p (h c) -> p h c", h=2)[:, :, 0:C],
                            func=AF.Exp,
                        )
                        eks[i] = ek
                        v2 = v2r[i % NVBUF]
                        nc.vector.tensor_copy(
                            v2[:].rearrange("p (h w) -> p h w", h=2)[:, :, 0:C],
                            psKV[:].rearrange("p (h c) -> p h c", h=2)[:, :, C : 2 * C],
                        )

                    for i in range(NSUPER):
                        psKV = kv_mms(i)
                        if i > 0:
                            ctx_mms(i - 1)
                        kv_post(i, psKV)
                    ctx_mms(NSUPER - 1)
                    # overlap the context epilogue with the first two Q chunks
                    # (their PSUM supertiles borrow the KV pool's slots)
                    for j in range(2):
                        psQ = pkv.tile([128, 1024], F32, name="psKV", tag="psKV")
                        q_mms_into(j, psQ)
                        eq_act(j, psQ)

                # ===== epilogue: normalize context, fold wo: W^T = ctx.T@woT =====
                rcol = [cst.tile([128, 1], F32, name=f"rcol{u}") for u in range(2)]
                ctx_sb = [cst.tile([128, C], MDT, name=f"ctx{u}") for u in range(2)]
                for u in range(2):
                    nc.vector.reciprocal(rcol[u][:], psCtx[u][:, C : C + 1])
                    nc.vector.tensor_scalar_mul(
                        out=ctx_sb[u][:], in0=psCtx[u][:, 0:C], scalar1=rcol[u][:]
                    )
                with tc.tile_pool(name="psum_w", bufs=1, space="PSUM") as pw:
                    psW = [pw.tile([128, C], F32, name=f"psW{v}") for v in range(2)]
                    for v in range(2):
                        for u in range(2):
                            nc.tensor.matmul(
                                psW[v][:],
                                ctx_sb[u][:, ts(v, 128)],
                                woT_sb[u],
                                start=(u == 0),
                                stop=(u == 1) and not use_bv,
                                skip_group_check=True,
                            )
                        if use_bv:
                            # context gains +bv[d'] per row (sum_n k = 1), so
                            # W^T += bv (X) rowsum(wo): a K=1 rank-1 matmul.
                            nc.tensor.matmul(
                                psW[v][:],
                                bv_sb[:, ts(v, 128)],
                                wosum_sb[:],
                                start=False,
                                stop=True,
                                skip_group_check=True,
                            )
                        nc.vector.tensor_copy(WT_sb[v][:], psW[v][:])

            # ============ Q phase: out = W^T.T @ softmax_head(exp(Q*s)) ============
            # Supertile layout [128, 1024]: channel-half t at cols 512t.
            # Pipelined depth 3: at iteration j the PE runs Q(j), D(j-1),
            # Rb(j-2), out(j-3) so every matmul's ACT/DVE inputs are a full
            # iteration old.
            with (
                tc.tile_pool(name="psq", bufs=1, space="PSUM") as pq,
                tc.tile_pool(name="psd", bufs=1, space="PSUM") as pd,
                tc.tile_pool(name="psrb", bufs=1, space="PSUM") as prb,
                tc.tile_pool(name="pso", bufs=1, space="PSUM") as po,
            ):
                def q_mms(j):
                    psQ = pq.tile([128, 1024], F32, name="psQ")
                    q_mms_into(j, psQ)
                    return psQ

                def d_mms(j):
                    psD = pd.tile([NHEADS, 512], F32, name="psD")
                    for t in range(2):
                        nc.tensor.matmul(
                            psD[:],
                            ind_sb[t],
                            eqs[j][:, t * 512 : (t + 1) * 512],
                            start=(t == 0),
                            stop=(t == 1),
                        )
                    psDs[j] = psD

                def r_acts(j):
                    lnD = dsb.tile([NHEADS, 512], F32, name="lnD")
                    nc.scalar.activation(out=lnD[:], in_=psDs.pop(j)[:], func=AF.Ln)
                    rD = dsb.tile([NHEADS, 512], MDT, name="rD")
                    nc.scalar.activation(out=rD[:], in_=lnD[:], func=AF.Exp, scale=-1.0)
                    rDs[j] = rD

                def rb_mms(j):
                    psRb = prb.tile([128, 1024], F32, name="psRb")
                    rD = rDs.pop(j)
                    for t in range(2):
                        nc.tensor.matmul(
                            psRb[:, t * 512 : (t + 1) * 512],
                            sel_sb[t][:],
                            rD[:],
                            start=True,
                            stop=True,
                        )
                    psRbs[j] = psRb

                def q_mul(j):
                    qt = qsb.tile([128, 1024], MDT, name="qt", tag="qt")
                    nc.vector.tensor_mul(qt[:], eqs.pop(j)[:], psRbs.pop(j)[:])
                    qts[j] = qt

                def out_mms(j):
                    psO = po.tile([128, 1024], F32, name="psO")
                    qt = qts.pop(j)
                    for t in range(2):
                        for u in range(2):
                            nc.tensor.matmul(
                                psO[:, t * 512 : (t + 1) * 512],
                                WT_sb[u][:, ts(t, 128)],
                                qt[:, u * 512 : (u + 1) * 512],
                                start=(u == 0),
                                stop=(u == 1),
                            )
                    psOs[j] = psO

                def store(j):
                    psO = psOs.pop(j)
                    o2 = qsb.tile([128, 1024], F32, name="o2", tag="o2")
                    if use_bo:
                        for t in range(2):
                            nc.vector.tensor_scalar_add(
                                out=o2[:, t * 512 : (t + 1) * 512],
                                in0=psO[:, t * 512 : (t + 1) * 512],
                                scalar1=bo_sb[t][:],
                            )
                    else:
                        nc.vector.tensor_copy(o2[:], psO[:])
                    for t in range(2):
                        nc.sync.dma_start(
                            out=y[ts(t, 128), ts(j, 512)],
                            in_=o2[:, t * 512 : (t + 1) * 512],
                        )

                for j in range(NCHUNKS + 3):
                    if 2 <= j < NCHUNKS:
                        psQ = q_mms(j)
                    if 1 <= j <= NCHUNKS:
                        d_mms(j - 1)
                    if 2 <= j <= NCHUNKS + 1:
                        rb_mms(j - 2)
                    if 3 <= j <= NCHUNKS + 2:
                        out_mms(j - 3)
                    if 2 <= j < NCHUNKS:
                        eq_act(j, psQ)
                    if 1 <= j <= NCHUNKS:
                        r_acts(j - 1)
                    if 2 <= j <= NCHUNKS + 1:
                        q_mul(j - 2)
                    if 3 <= j <= NCHUNKS + 2:
                        store(j - 3)

    nc.compile()
    return nc


def _get_nc(use_bq, use_bo, use_bv, mm_dtype):
    key = (use_bq, use_bo, use_bv, str(mm_dtype))
    if key not in _CACHE:
        with _single_act_table():
            _CACHE[key] = _build(use_bq, use_bo, use_bv, mm_dtype)
    return _CACHE[key]


def kernel(x, cproj, wq, bq, wkv, bkv, wo, bo, _mm_dtype=F32R, _results_hook=None):
    x = np.ascontiguousarray(np.asarray(x, dtype=np.float32).reshape(B, C, N))
    cf = np.ascontiguousarray(np.asarray(cproj, dtype=np.float32).reshape(B, C, N))
    wq = np.asarray(wq, dtype=np.float32)
    wkv = np.asarray(wkv, dtype=np.float32)
    wo = np.asarray(wo, dtype=np.float32)
    bq = np.asarray(bq, dtype=np.float32)
    bkv = np.asarray(bkv, dtype=np.float32)
    bo = np.asarray(bo, dtype=np.float32)

    use_bq = bool(np.any(bq != 0))
    use_bo = bool(np.any(bo != 0))
    bv = bkv[C:]
    use_bv = bool(np.any(bv != 0))

    wqT = np.ascontiguousarray(wq.T)
    wkvT = np.ascontiguousarray(wkv.T)
    woT = np.ascontiguousarray(wo.T)
    ind = np.zeros((C, NHEADS), np.float32)
    ind[np.arange(C), np.arange(C) // DHEAD] = 1.0
    sel = np.ascontiguousarray(ind.T)

    # packed weights: per c-half u: [wkvT | wqT | woT | ind] along the free dim
    wpack = np.zeros((128, 2 * WP), np.float32)
    for u in range(2):
        r = slice(u * 128, (u + 1) * 128)
        wpack[:, u * WP : u * WP + 2 * C] = wkvT[r]
        wpack[:, u * WP + 2 * C : u * WP + 3 * C] = wqT[r]
        wpack[:, u * WP + 3 * C : u * WP + 4 * C] = woT[r]
        wpack[:, u * WP + 4 * C : u * WP + 4 * C + NHEADS] = ind[r]

    nc = _get_nc(use_bq, use_bo, use_bv, _mm_dtype)

    base = {"wpack": wpack, "sel": sel}
    if use_bq:
        base["bq_s"] = (SCALE * bq).reshape(C, 1)
    if use_bo:
        base["bo_c"] = bo.reshape(C, 1)
    if use_bv:
        base["bv_r"] = bv.reshape(1, C)
        base["wosum"] = wo.sum(axis=1).reshape(1, C)

    in_maps = [dict(base, x=x[b], cp=cf[b]) for b in range(B)]
    res = run_bass_kernel_spmd(nc, in_maps, list(range(NCORES)))
    if _results_hook is not None:
        _results_hook(res)
    out = np.stack([res.results[b]["y"] for b in range(B)], axis=0)
    return out.reshape(B, C, H, W)


# revision 21
# speedup vs baseline: 2.6524x; 1.0125x over previous
"""Trainium2 Bass kernel for CrossEfficientAttention (B=8, C=256, H=W=64, 4 heads).

Sharding: data-parallel over batch B — one sample per NeuronCore, no collectives.

Per-core math (sample x_s, c_s of shape [C, N], N = H*W = 4096):
    Q  = wq @ x_s                      (+ bq, folded into the exp's ACT bias)
    KV = wkv @ c_s                     (bkv[:C] cancels exactly in softmax over N;
                                        bkv[C:] handled as a rank-1 update of W)
    k  = softmax_N(K); q = softmax_head(Q * C**-0.25)
    context = k @ V^T ; out = wo @ (context @ q) + bo

Restructured for the PE array (out = lhsT.T @ rhs, contraction over partitions):
  * KV^T computed directly in [N, C] layout by using c_s tiles as lhsT.
  * k-softmax normalizer: ones-columns appended to V^T give row sums of exp(K)
    in column 256 of the context PSUM accumulator; context rows are then scaled
    by the reciprocal column (per-partition tensor_scalar) — no transposes.
  * wo folded in early: W^T = matmul(lhsT=context, rhs=wo^T) directly in [d, o]
    layout. The per-chunk output is then just out2 = W^T.T @ q.
  * q-softmax denominators: block-indicator matmul sums exp(Q) per head into a
    [4, 512] PSUM tile; 1/D = exp(-ln D) on ScalarE (vector.reciprocal is
    8 cyc/elem, ACT Reciprocal is blocked); broadcast back to 128 partitions
    with a tiny selector matmul.

Matmuls run in float32r (single-pass PE, 4x faster than fp32 emulation).
Both loops are explicitly software-pipelined so the in-order PE queue never
waits on the ACT/DVE stages of the same iteration; weights ride in a single
packed DMA and output stores use the scalar-engine HWDGE queue to keep the
sync queue free for input streaming.
"""

import numpy as np

import concourse.bass as bass
import concourse.tile as tile
from concourse import bacc, mybir
from concourse.bass import ts
from concourse.bass_utils import run_bass_kernel_spmd

B, C, H, W = 8, 256, 64, 64
N = H * W
NHEADS = 4
DHEAD = C // NHEADS
NCORES = 8
NSUPER = N // 256          # 16 double-n-tile iterations for the KV phase
NCHUNKS = N // 512         # 8 column chunks for the Q/output phase
SCALE = float(1.0 / np.sqrt(np.sqrt(np.float32(C))))
VW = C + 2                 # V^T tile row width (256 data + 2 ones cols)
WP = 2 * C + C + C + NHEADS  # packed weight row width per c-half: wkvT|wqT|woT|ind

F32 = mybir.dt.float32
F32R = mybir.dt.float32r
AF = mybir.ActivationFunctionType

_CACHE = {}


def _single_act_table():
    """Scope-patch the activation-table list so the table-load pass resolves
    both Exp and Ln to natural_log_exp_and_others (set ids stay positional,
    so only the function lists may change, not the order)."""
    import contextlib

    import concourse.bacc as cbacc
    from concourse.hw_specs import get_activation_tables

    @contextlib.contextmanager
    def scope():
        orig = cbacc.get_activation_tables

        def patched(arch):
            tabs = get_activation_tables(arch)
            return {
                k: (v if k == "natural_log_exp_and_others" else set())
                for k, v in tabs.items()
            }

        cbacc.get_activation_tables = patched
        try:
            yield
        finally:
            cbacc.get_activation_tables = orig

    return scope()


def _build(use_bq, use_bo, use_bv, mm_dtype):
    nc = bacc.Bacc("TRN2", target_bir_lowering=False, debug=False)
    MDT = mm_dtype

    x = nc.dram_tensor("x", [C, N], MDT, kind="ExternalInput")
    cp = nc.dram_tensor("cp", [C, N], MDT, kind="ExternalInput")
    wpack = nc.dram_tensor("wpack", [128, 2 * WP], MDT, kind="ExternalInput")
    sel = nc.dram_tensor("sel", [NHEADS, C], MDT, kind="ExternalInput")
    if use_bq:
        bq_s = nc.dram_tensor("bq_s", [C, 1], F32, kind="ExternalInput")
    if use_bo:
        bo_c = nc.dram_tensor("bo_c", [C, 1], F32, kind="ExternalInput")
    if use_bv:
        bv_r = nc.dram_tensor("bv_r", [1, C], MDT, kind="ExternalInput")
        wosum = nc.dram_tensor("wosum", [1, C], MDT, kind="ExternalInput")
    y = nc.dram_tensor("y", [C, N], F32, kind="ExternalOutput")

    with tile.TileContext(nc) as tc:
        with (
            tc.tile_pool(name="const", bufs=1) as cst,
            tc.tile_pool(name="big", bufs=1) as big,
            tc.tile_pool(name="qsb", bufs=4) as qsb,
            tc.tile_pool(name="dsb", bufs=3) as dsb,
        ):
            # --- packed weights; the KV-phase slice (wkvT) rides first ---
            wpk = cst.tile([128, 2 * WP], MDT, name="wpk")
            wpk3 = wpk[:].rearrange("p (u w) -> p u w", u=2)
            wpack3 = wpack[:].rearrange("p (u w) -> p u w", u=2)
            wkvT_sb = [wpk[:, u * WP : u * WP + 2 * C] for u in range(2)]
            wqT_sb = [wpk[:, u * WP + 2 * C : u * WP + 3 * C] for u in range(2)]
            woT_sb = [wpk[:, u * WP + 3 * C : u * WP + 4 * C] for u in range(2)]
            ind_sb = [wpk[:, u * WP + 4 * C : u * WP + 4 * C + NHEADS] for u in range(2)]
            sel_sb = [cst.tile([NHEADS, 128], MDT, name=f"sel{u}") for u in range(2)]
            for u in range(2):
                nc.scalar.dma_start(out=sel_sb[u][:], in_=sel[:, ts(u, 128)])
            if use_bq:
                bq_sb = [cst.tile([128, 1], F32, name=f"bq{u}") for u in range(2)]
                for u in range(2):
                    nc.scalar.dma_start(out=bq_sb[u][:], in_=bq_s[ts(u, 128), :])
            if use_bo:
                bo_sb = [cst.tile([128, 1], F32, name=f"bo{u}") for u in range(2)]
                for u in range(2):
                    nc.scalar.dma_start(out=bo_sb[u][:], in_=bo_c[ts(u, 128), :])
            if use_bv:
                bv_sb = cst.tile([1, C], MDT, name="bv_sb")
                nc.scalar.dma_start(out=bv_sb[:], in_=bv_r[:])
                wosum_sb = cst.tile([1, C], MDT, name="wosum_sb")
                nc.scalar.dma_start(out=wosum_sb[:], in_=wosum[:])

            # --- sample loads: gate the first KV iterations on as little
            # data as possible, then stream the rest just ahead of use ---
            cf_sb = [big.tile([128, N], MDT, name=f"cf{u}") for u in range(2)]
            for u in range(2):
                nc.sync.dma_start(out=cf_sb[u][:, 0:512], in_=cp[ts(u, 128), 0:512])
            nc.sync.dma_start(out=wpk3[:, :, 0 : 2 * C], in_=wpack3[:, :, 0 : 2 * C])
            for c0, c1 in ((512, 1536), (1536, 2560), (2560, 3584), (3584, 4096)):
                for u in range(2):
                    nc.sync.dma_start(
                        out=cf_sb[u][:, c0:c1], in_=cp[ts(u, 128), c0:c1]
                    )
            nc.sync.dma_start(out=wpk3[:, :, 2 * C : WP], in_=wpack3[:, :, 2 * C : WP])
            xf_sb = [big.tile([128, N], MDT, name=f"xf{u}") for u in range(2)]
            for c0, c1 in ((0, 2048), (2048, 4096)):
                for u in range(2):
                    nc.sync.dma_start(
                        out=xf_sb[u][:, c0:c1], in_=x[ts(u, 128), c0:c1]
                    )

            # persistent W^T tiles (filled in the epilogue)
            WT_sb = [cst.tile([128, C], MDT, name=f"WT{u}") for u in range(2)]

            # HAM warmup: ~10 dependency-free matmuls on scratch data keep the
            # PE busy during the initial DMA wait so real matmuls start at
            # K=8/8 (2.4 GHz) instead of ramping from 1.2 GHz.
            scratch = cst.tile([128, 512], MDT, name="scratch")
            nc.vector.memset(scratch[:].bitcast(F32), 1.0)
            with tc.tile_pool(name="pswarm", bufs=1, space="PSUM") as pwm:
                pswarm = pwm.tile([128, 512], F32, name="pswarm")
                for _ in range(22):
                    nc.tensor.matmul(
                        pswarm[:], scratch[:, 0:128], scratch[:],
                        start=True, stop=True, skip_group_check=True,
                    )

            # manually-rotated V^T ring: ones columns pre-set once
            NVBUF = 4
            v2r = [cst.tile([128, 2 * VW], MDT, name=f"v2_{i}") for i in range(NVBUF)]
            for i in range(NVBUF):
                for h in range(2):
                    nc.vector.memset(
                        v2r[i][:, h * VW + C : h * VW + C + 2].bitcast(F32), 1.0
                    )

            eqs, psDs, rDs, psRbs, qts, psOs = {}, {}, {}, {}, {}, {}

            def q_mms_into(j, psQ):
                for t in range(2):
                    for u in range(2):
                        nc.tensor.matmul(
                            psQ[:, t * 512 : (t + 1) * 512],
                            wqT_sb[u][:, ts(t, 128)],
                            xf_sb[u][:, ts(j, 512)],
                            start=(u == 0),
                            stop=(u == 1),
                        )

            def eq_act(j, psQ):
                eq = qsb.tile([128, 1024], MDT, name="eq", tag="eq")
                if use_bq:
                    for t in range(2):
                        nc.scalar.activation(
                            out=eq[:, t * 512 : (t + 1) * 512],
                            in_=psQ[:, t * 512 : (t + 1) * 512],
                            func=AF.Exp,
                            scale=SCALE,
                            bias=bq_sb[t][:],
                        )
                else:
                    nc.scalar.activation(
                        out=eq[:], in_=psQ[:], func=AF.Exp, scale=SCALE
                    )
                eqs[j] = eq

            # ============ KV phase: context = exp(K) @ [V^T | 1] ============
            # Software-pipelined by one iteration: the PE runs iteration i's
            # KV matmuls and iteration i-1's context matmuls back to back.
            with tc.tile_pool(name="psum_ctx", bufs=1, space="PSUM") as pctx:
                psCtx = [
                    pctx.tile([128, C + 2], F32, name=f"psCtx{u}") for u in range(2)
                ]
                with (
                    tc.tile_pool(name="psum_kv", bufs=3, space="PSUM") as pkv,
                    tc.tile_pool(name="kvsb", bufs=3) as kvsb,
                ):
                    eks = {}

                    def kv_mms(i):
                        psKV = pkv.tile([128, 1024], F32, name="psKV")
                        for h in range(2):
                            nt = 2 * i + h
                            for u in range(2):
                                nc.tensor.matmul(
                                    psKV[:, h * 512 : (h + 1) * 512],
                                    cf_sb[u][:, ts(nt, 128)],
                                    wkvT_sb[u],
                                    start=(u == 0),
                                    stop=(u == 1),
                                )
                        return psKV

                    def ctx_mms(i):
                        ek = eks.pop(i)
                        v2 = v2r[i % NVBUF]
                        for h in range(2):
                            for u in range(2):
                                nc.tensor.matmul(
                                    psCtx[u][:],
                                    ek[:, h, ts(u, 128)],
                                    v2[:, h * VW : (h + 1) * VW],
                                    start=(i == 0 and h == 0),
                                    stop=(i == NSUPER - 1 and h == 1),
                                    skip_group_check=True,
                                )

                    def kv_post(i, psKV):
                        ek = kvsb.tile([128, 2, C], MDT, name="ek")
                        nc.scalar.activation(
                            out=ek[:],
                            in_=psKV[:].rearrange("p (h c) -> p h c", h=2)[:, :, 0:C],
                            func=AF.Exp,
                        )
                        eks[i] = ek
                        v2 = v2r[i % NVBUF]
                        nc.vector.tensor_copy(
                            v2[:].rearrange("p (h w) -> p h w", h=2)[:, :, 0:C],
                            psKV[:].rearrange("p (h c) -> p h c", h=2)[:, :, C : 2 * C],
                        )

                    for i in range(NSUPER):
                        psKV = kv_mms(i)
                        if i > 0:
                            ctx_mms(i - 1)
                        kv_post(i, psKV)
                    ctx_mms(NSUPER - 1)
                    # overlap the context epilogue with the first two Q chunks
                    # (their PSUM supertiles borrow the KV pool's slots)
                    for j in range(2):
                        psQ = pkv.tile([128, 1024], F32, name="psKV", tag="psKV")
                        q_mms_into(j, psQ)
                        eq_act(j, psQ)

                # ===== epilogue: normalize context, fold wo: W^T = ctx.T@woT =====
                rcol = [cst.tile([128, 1], F32, name=f"rcol{u}") for u in range(2)]
                ctx_sb = [cst.tile([128, C], MDT, name=f"ctx{u}") for u in range(2)]
                for u in range(2):
                    nc.vector.reciprocal(rcol[u][:], psCtx[u][:, C : C + 1])
                    nc.vector.tensor_scalar_mul(
                        out=ctx_sb[u][:], in0=psCtx[u][:, 0:C], scalar1=rcol[u][:]
                    )
                with tc.tile_pool(name="psum_w", bufs=1, space="PSUM") as pw:
                    psW = [pw.tile([128, C], F32, name=f"psW{v}") for v in range(2)]
                    for v in range(2):
                        for u in range(2):
                            nc.tensor.matmul(
                                psW[v][:],
                                ctx_sb[u][:, ts(v, 128)],
                                woT_sb[u],
                                start=(u == 0),
                                stop=(u == 1) and not use_bv,
                                skip_group_check=True,
                            )
                        if use_bv:
                            # context gains +bv[d'] per row (sum_n k = 1), so
                            # W^T += bv (X) rowsum(wo): a K=1 rank-1 matmul.
                            nc.tensor.matmul(
                                psW[v][:],
                                bv_sb[:, ts(v, 128)],
                                wosum_sb[:],
                                start=False,
                                stop=True,
                                skip_group_check=True,
                            )
                        nc.vector.tensor_copy(WT_sb[v][:], psW[v][:])

            # ============ Q phase: out = W^T.T @ softmax_head(exp(Q*s)) ============
            # Supertile layout [128, 1024]: channel-half t at cols 512t.
            # Pipelined depth 3: at iteration j the PE runs Q(j), D(j-1),
            # Rb(j-2), out(j-3) so every matmul's ACT/DVE inputs are a full
            # iteration old.
            with (
                tc.tile_pool(name="psq", bufs=1, space="PSUM") as pq,
                tc.tile_pool(name="psd", bufs=1, space="PSUM") as pd,
                tc.tile_pool(name="psrb", bufs=1, space="PSUM") as prb,
                tc.tile_pool(name="pso", bufs=1, space="PSUM") as po,
            ):
                def q_mms(j):
                    psQ = pq.tile([128, 1024], F32, name="psQ")
                    q_mms_into(j, psQ)
                    return psQ

                def d_mms(j):
                    psD = pd.tile([NHEADS, 512], F32, name="psD")
                    for t in range(2):
                        nc.tensor.matmul(
                            psD[:],
                            ind_sb[t],
                            eqs[j][:, t * 512 : (t + 1) * 512],
                            start=(t == 0),
                            stop=(t == 1),
                        )
                    psDs[j] = psD

                def r_acts(j):
                    lnD = dsb.tile([NHEADS, 512], F32, name="lnD")
                    nc.scalar.activation(out=lnD[:], in_=psDs.pop(j)[:], func=AF.Ln)
                    rD = dsb.tile([NHEADS, 512], MDT, name="rD")
                    nc.scalar.activation(out=rD[:], in_=lnD[:], func=AF.Exp, scale=-1.0)
                    rDs[j] = rD

                def rb_mms(j):
                    psRb = prb.tile([128, 1024], F32, name="psRb")
                    rD = rDs.pop(j)
                    for t in range(2):
                        nc.tensor.matmul(
                            psRb[:, t * 512 : (t + 1) * 512],
                            sel_sb[t][:],
                            rD[:],
                            start=True,
                            stop=True,
                        )
                    psRbs[j] = psRb

                def q_mul(j):
                    qt = qsb.tile([128, 1024], MDT, name="qt", tag="qt")
                    nc.vector.tensor_mul(qt[:], eqs.pop(j)[:], psRbs.pop(j)[:])
                    qts[j] = qt

                def out_mms(j):
                    psO = po.tile([128, 1024], F32, name="psO")
                    qt = qts.pop(j)
                    for t in range(2):
                        for u in range(2):
                            nc.tensor.matmul(
                                psO[:, t * 512 : (t + 1) * 512],
                                WT_sb[u][:, ts(t, 128)],
                                qt[:, u * 512 : (u + 1) * 512],
                                start=(u == 0),
                                stop=(u == 1),
                            )
                    psOs[j] = psO

                def store(j):
                    psO = psOs.pop(j)
                    o2 = qsb.tile([128, 1024], F32, name="o2", tag="o2")
                    if use_bo:
                        for t in range(2):
                            nc.vector.tensor_scalar_add(
                                out=o2[:, t * 512 : (t + 1) * 512],
                                in0=psO[:, t * 512 : (t + 1) * 512],
                                scalar1=bo_sb[t][:],
                            )
                    else:
                        nc.vector.tensor_copy(o2[:], psO[:])
                    for t in range(2):
                        nc.sync.dma_start(
                            out=y[ts(t, 128), ts(j, 512)],
                            in_=o2[:, t * 512 : (t + 1) * 512],
                        )

                for j in range(NCHUNKS + 3):
                    if 2 <= j < NCHUNKS:
                        psQ = q_mms(j)
                    if 1 <= j <= NCHUNKS:
                        d_mms(j - 1)
                    if 2 <= j <= NCHUNKS + 1:
                        rb_mms(j - 2)
                    if 3 <= j <= NCHUNKS + 2:
                        out_mms(j - 3)
                    if 2 <= j < NCHUNKS:
                        eq_act(j, psQ)
                    if 1 <= j <= NCHUNKS:
                        r_acts(j - 1)
                    if 2 <= j <= NCHUNKS + 1:
                        q_mul(j - 2)
                    if 3 <= j <= NCHUNKS + 2:
                        store(j - 3)

    nc.compile()
    return nc


def _get_nc(use_bq, use_bo, use_bv, mm_dtype):
    key = (use_bq, use_bo, use_bv, str(mm_dtype))
    if key not in _CACHE:
        with _single_act_table():
            _CACHE[key] = _build(use_bq, use_bo, use_bv, mm_dtype)
    return _CACHE[key]


def kernel(x, cproj, wq, bq, wkv, bkv, wo, bo, _mm_dtype=F32R, _results_hook=None):
    x = np.ascontiguousarray(np.asarray(x, dtype=np.float32).reshape(B, C, N))
    cf = np.ascontiguousarray(np.asarray(cproj, dtype=np.float32).reshape(B, C, N))
    wq = np.asarray(wq, dtype=np.float32)
    wkv = np.asarray(wkv, dtype=np.float32)
    wo = np.asarray(wo, dtype=np.float32)
    bq = np.asarray(bq, dtype=np.float32)
    bkv = np.asarray(bkv, dtype=np.float32)
    bo = np.asarray(bo, dtype=np.float32)

    use_bq = bool(np.any(bq != 0))
    use_bo = bool(np.any(bo != 0))
    bv = bkv[C:]
    use_bv = bool(np.any(bv != 0))

    wqT = np.ascontiguousarray(wq.T)
    wkvT = np.ascontiguousarray(wkv.T)
    woT = np.ascontiguousarray(wo.T)
    ind = np.zeros((C, NHEADS), np.float32)
    ind[np.arange(C), np.arange(C) // DHEAD] = 1.0
    sel = np.ascontiguousarray(ind.T)

    # packed weights: per c-half u: [wkvT | wqT | woT | ind] along the free dim
    wpack = np.zeros((128, 2 * WP), np.float32)
    for u in range(2):
        r = slice(u * 128, (u + 1) * 128)
        wpack[:, u * WP : u * WP + 2 * C] = wkvT[r]
        wpack[:, u * WP + 2 * C : u * WP + 3 * C] = wqT[r]
        wpack[:, u * WP + 3 * C : u * WP + 4 * C] = woT[r]
        wpack[:, u * WP + 4 * C : u * WP + 4 * C + NHEADS] = ind[r]

    nc = _get_nc(use_bq, use_bo, use_bv, _mm_dtype)

    base = {"wpack": wpack, "sel": sel}
    if use_bq:
        base["bq_s"] = (SCALE * bq).reshape(C, 1)
    if use_bo:
        base["bo_c"] = bo.reshape(C, 1)
    if use_bv:
        base["bv_r"] = bv.reshape(1, C)
        base["wosum"] = wo.sum(axis=1).reshape(1, C)

    in_maps = [dict(base, x=x[b], cp=cf[b]) for b in range(B)]
    res = run_bass_kernel_spmd(nc, in_maps, list(range(NCORES)))
    if _results_hook is not None:
        _results_hook(res)
    out = np.stack([res.results[b]["y"] for b in range(B)], axis=0)
    return out.reshape(B, C, H, W)
